# revision 13
# baseline (speedup 1.0000x reference)
"""Trainium2 Bass kernel for nn_Attention_46901042872408.

Dense MHA transformer block with RoPE + prefix-tuning branch:
  q/k/v = x @ wq/wk/wv; rope(q), rope(k); causal attention;
  prefix branch: non-causal attention of q against (prefix @ wk/wv),
  gated by tanh(prefix_gate) per head; out = (attn + gate*prefix_attn) @ wo.

Sharding: 8 cores = data-parallel over batch (2) x tensor-parallel over
heads (4 groups of 8 heads). Each core computes a partial [2048, 4096]
output (its heads' contribution through its wo row-slice); host sums the
4 partials per batch.

v3 design: all matmul operands bf16 (1 cyc/row, half DMA traffic).
k/v/attT SBUF-resident; q spilled to DRAM in phase 1 and streamed back
once per 512-token q-block (contiguous reads). Phase 2 runs q-block
outer / head inner, and phase 3 (out = attT.T @ wo) is fused per
q-block right after its 8 heads finish, so phase-3 matmuls fill phase-2
dependency stalls. No inter-phase barriers.

Per-core pipeline:
  Phase 1: x streamed in 384-token chunks (double-buffered), wqkv
    streamed in 256-col blocks per chunk; psum [tok, col] projections;
    RoPE via resident bf16 cos/sin tables; q/k PE-transposed ([hd, tok]),
    k into resident kT, q spilled; v copied straight into resident v
    [tok, hd]. Prefix k/v projections ride the chunk-0 weight stream.
  Phase 2 (per q-block, per head): scoresT [k_tok, q_tok] tiles, exp on
    ACT (-> bf16), causal mask multiply on diagonal tiles, PV + ones-
    denominator matmuls accumulated in PSUM, prefix branch with 30
    k-rows, combine via reciprocal + gpsimd partition-broadcast into
    resident attT.
  Phase 3 (fused per q-block): out rows = attT.T @ wo with wo streamed
    per 512-col output block; direct DMA of [128, 512] f32 tiles.
"""

import os
import sys

sys.path.insert(0, "/opt/trn_rl_repo")

import numpy as np

B, S, D = 2, 2048, 4096
H, HD = 32, 128
PFX = 30
NCORES = 8
CPB = 4  # cores per batch (head-parallel groups)
HPC = 8  # heads per core
COLS = HPC * HD  # 1024 qkv columns per core
WB = 256  # weight column-block
NWB = 3 * COLS // WB  # 12
NKT = D // 128  # 32 contraction tiles
TCH = 384  # max token chunk for phase 1 (sized by SBUF)
CHUNKS = [(tb, min(TCH, S - tb)) for tb in range(0, S, TCH)]
SCALE = 1.0 / float(np.sqrt(HD))

_CACHE = {}


def _build(mm_fast=True):
    from contextlib import ExitStack

    def knob(name, default):
        return int(os.environ.get(name, default))

    import concourse.tile as tile
    from concourse import bacc, mybir

    f32 = mybir.dt.float32
    mdt = mybir.dt.bfloat16 if mm_fast else mybir.dt.float32
    AF = mybir.ActivationFunctionType
    OP = mybir.AluOpType

    nc = bacc.Bacc("TRN2", target_bir_lowering=False, debug=False, num_devices=NCORES)

    xT = nc.dram_tensor("xT", [D, S], mdt, kind="ExternalInput")
    wqkv = nc.dram_tensor("wqkv", [D, 3 * COLS], mdt, kind="ExternalInput")
    wo_d = nc.dram_tensor("wo", [COLS, D], mdt, kind="ExternalInput")
    pfT = nc.dram_tensor("pfT", [D, PFX], mdt, kind="ExternalInput")
    cosS = nc.dram_tensor("cosS", [S, 128], mdt, kind="ExternalInput")
    sinS = nc.dram_tensor("sinS", [S, 128], mdt, kind="ExternalInput")
    masks = nc.dram_tensor("masks", [128, 4, 512], mdt, kind="ExternalInput")
    ones_d = nc.dram_tensor("ones", [128, 1], mdt, kind="ExternalInput")
    eye_d = nc.dram_tensor("eye", [128, 128], mdt, kind="ExternalInput")
    g_d = nc.dram_tensor("g", [1, HPC], f32, kind="ExternalInput")
    out_d = nc.dram_tensor("out", [S, D], f32, kind="ExternalOutput")

    with tile.TileContext(nc) as tc:
        with ExitStack() as top:
            dram = top.enter_context(tc.tile_pool(name="dram", bufs=1, space="DRAM"))
            q_sp = dram.tile([COLS, S], mdt)

            pres = top.enter_context(tc.tile_pool(name="res", bufs=1))
            eye_sb = pres.tile([128, 128], mdt)
            nc.sync.dma_start(eye_sb[:], eye_d[:])
            ones_sb = pres.tile([128, 1], mdt)
            nc.sync.dma_start(ones_sb[:], ones_d[:])
            g_sb = pres.tile([1, HPC], f32)
            nc.sync.dma_start(g_sb[:], g_d[:])

            # residents spanning all phases
            pqkv = top.enter_context(tc.tile_pool(name="pqkv", bufs=1))
            kT_sb = pqkv.tile([128, HPC, S], mdt)
            v_sb = pqkv.tile([128, S // 128, COLS], mdt)
            attT_sb = pqkv.tile([128, HPC, S], mdt)
            pkT_sb = pqkv.tile([128, HPC, PFX], mdt)
            pv_sb = pqkv.tile([PFX, COLS], mdt)

            # ---------------- Phase 1: projections ----------------
            with ExitStack() as ph1:
                px = ph1.enter_context(tc.tile_pool(name="px", bufs=knob("B_px", 2)))
                pw = ph1.enter_context(tc.tile_pool(name="pw", bufs=knob("B_pw", 2)))
                pcs = ph1.enter_context(tc.tile_pool(name="pcs", bufs=1))
                ptmp = ph1.enter_context(
                    tc.tile_pool(name="ptmp", bufs=knob("B_ptmp", 2))
                )
                po = ph1.enter_context(tc.tile_pool(name="po", bufs=knob("B_po", 3)))
                poT = ph1.enter_context(tc.tile_pool(name="poT", bufs=knob("B_poT", 3)))
                ppk = ph1.enter_context(tc.tile_pool(name="ppk", bufs=2))
                ps_mm = ph1.enter_context(
                    tc.tile_pool(name="ps_mm", bufs=knob("B_psmm", 3), space="PSUM")
                )
                ps_tr = ph1.enter_context(
                    tc.tile_pool(name="ps_tr", bufs=knob("B_pstr", 2), space="PSUM")
                )
                ps_pk = ph1.enter_context(
                    tc.tile_pool(name="ps_pk", bufs=1, space="PSUM")
                )
                ps_ptr = ph1.enter_context(
                    tc.tile_pool(name="ps_ptr", bufs=1, space="PSUM")
                )

                # resident rope tables [tok_part, mt, 128]
                cos_sb = pcs.tile([128, S // 128, 128], mdt)
                nc.sync.dma_start(
                    cos_sb[:], cosS[:].rearrange("(m p) j -> p m j", p=128)
                )
                sin_sb = pcs.tile([128, S // 128, 128], mdt)
                nc.sync.dma_start(
                    sin_sb[:], sinS[:].rearrange("(m p) j -> p m j", p=128)
                )
                # resident prefix xT
                pf_sb = pcs.tile([128, NKT, PFX], mdt)
                nc.sync.dma_start(
                    pf_sb[:], pfT[:].rearrange("(ko p) n -> p ko n", p=128)
                )

                for ck, (tb, ntok) in enumerate(CHUNKS):
                    x_sb = px.tile([128, NKT, ntok], mdt, tag="x")
                    nc.sync.dma_start(
                        x_sb[:],
                        xT[:, tb : tb + ntok].rearrange("(ko p) n -> p ko n", p=128),
                    )
                    for wb in range(NWB):
                        w_sb = pw.tile([128, NKT, WB], mdt, tag="w")
                        nc.sync.dma_start(
                            w_sb[:],
                            wqkv[:, wb * WB : (wb + 1) * WB].rearrange(
                                "(ko p) c -> p ko c", p=128
                            ),
                        )
                        if ck == 0 and wb >= 4:
                            # prefix projections off the same weight stream
                            psp = ps_pk.tile([PFX, WB], f32, tag="ppk")
                            for ki in range(NKT):
                                nc.tensor.matmul(
                                    psp[:],
                                    lhsT=pf_sb[:, ki, :],
                                    rhs=w_sb[:, ki, :],
                                    start=(ki == 0),
                                    stop=(ki == NKT - 1),
                                )
                            if wb < 8:  # k-cols -> pkT (transposed per head)
                                pks = ppk.tile([PFX, WB], mdt, tag="pks")
                                nc.scalar.activation(pks[:], psp[:], AF.Copy)
                                for c in range(2):
                                    h = (wb - 4) * 2 + c
                                    ptr = ps_ptr.tile([128, PFX], mdt, tag="ptr")
                                    nc.tensor.transpose(
                                        ptr[:],
                                        pks[:, c * 128 : (c + 1) * 128],
                                        eye_sb[0:PFX, 0:PFX],
                                    )
                                    nc.vector.tensor_copy(pkT_sb[:, h, :], ptr[:])
                            else:  # v-cols -> straight prefix-v
                                nc.scalar.activation(
                                    pv_sb[:, (wb - 8) * WB : (wb - 7) * WB],
                                    psp[:],
                                    AF.Copy,
                                )
                        for mt in range(ntok // 128):
                            tok0 = tb + mt * 128
                            gmt = tok0 // 128
                            ps = ps_mm.tile([128, WB], f32, tag="mm")
                            for ki in range(NKT):
                                nc.tensor.matmul(
                                    ps[:],
                                    lhsT=x_sb[:, ki, mt * 128 : (mt + 1) * 128],
                                    rhs=w_sb[:, ki, :],
                                    start=(ki == 0),
                                    stop=(ki == NKT - 1),
                                )
                            if wb < 8:  # q/k: rope, transpose
                                p3 = ps[:].rearrange("p (i two) -> p i two", two=2)
                                o = po.tile([128, WB], mdt, tag="o")
                                o3 = o[:].rearrange("p (i two) -> p i two", two=2)
                                cc = cos_sb[:, gmt, :]
                                ss = sin_sb[:, gmt, :]
                                m1 = ptmp.tile([128, 128], f32, tag="m1")
                                m2 = ptmp.tile([128, 128], f32, tag="m2")
                                nc.vector.tensor_tensor(m1[:], p3[:, :, 0], cc, OP.mult)
                                nc.vector.tensor_tensor(m2[:], p3[:, :, 1], ss, OP.mult)
                                nc.vector.tensor_tensor(
                                    o3[:, :, 0], m1[:], m2[:], OP.subtract
                                )
                                m3 = ptmp.tile([128, 128], f32, tag="m1")
                                m4 = ptmp.tile([128, 128], f32, tag="m2")
                                nc.vector.tensor_tensor(m3[:], p3[:, :, 0], ss, OP.mult)
                                nc.vector.tensor_tensor(m4[:], p3[:, :, 1], cc, OP.mult)
                                nc.vector.tensor_tensor(
                                    o3[:, :, 1], m3[:], m4[:], OP.add
                                )
                                for c in range(2):
                                    hh = (wb % 4) * 2 + c
                                    ptr2 = ps_tr.tile([128, 128], mdt, tag="tr")
                                    nc.tensor.transpose(
                                        ptr2[:],
                                        o[:, c * 128 : (c + 1) * 128],
                                        eye_sb[:],
                                    )
                                    if wb < 4:  # q: spill to DRAM
                                        oT = poT.tile([128, 128], mdt, tag="oT")
                                        nc.scalar.activation(oT[:], ptr2[:], AF.Copy)
                                        row0 = hh * 128
                                        nc.sync.dma_start(
                                            q_sp[row0 : row0 + 128, tok0 : tok0 + 128],
                                            oT[:],
                                        )
                                    else:  # k: resident
                                        nc.scalar.activation(
                                            kT_sb[:, hh, tok0 : tok0 + 128],
                                            ptr2[:],
                                            AF.Copy,
                                        )
                            else:  # v: copy straight into resident
                                col0 = (wb - 8) * WB
                                nc.scalar.activation(
                                    v_sb[:, gmt, col0 : col0 + WB], ps[:], AF.Copy
                                )

            # ---------------- Phases 2+3 fused, per q-block ----------------
            with ExitStack() as ph2:
                pmask = ph2.enter_context(tc.tile_pool(name="pmask", bufs=1))
                masks_sb = pmask.tile([128, 4, 512], mdt)
                nc.sync.dma_start(masks_sb[:], masks[:])
                pq = ph2.enter_context(tc.tile_pool(name="pq", bufs=knob("B_pq", 2)))
                pwo = ph2.enter_context(tc.tile_pool(name="pwo", bufs=knob("B_pwo", 2)))
                pE = ph2.enter_context(tc.tile_pool(name="pE", bufs=knob("B_pE", 18)))
                pc = ph2.enter_context(tc.tile_pool(name="pc", bufs=knob("B_pc", 3)))
                pout = ph2.enter_context(
                    tc.tile_pool(name="pout", bufs=knob("B_pout", 3))
                )
                ps_s = ph2.enter_context(
                    tc.tile_pool(name="ps_s", bufs=knob("B_pss", 2), space="PSUM")
                )
                ps_pv = ph2.enter_context(
                    tc.tile_pool(name="ps_pv", bufs=knob("B_ppv", 2), space="PSUM")
                )
                ps_den = ph2.enter_context(
                    tc.tile_pool(name="ps_den", bufs=knob("B_pden", 1), space="PSUM")
                )
                ps3 = ph2.enter_context(
                    tc.tile_pool(name="ps3", bufs=knob("B_ps3", 3), space="PSUM")
                )

                for qb in range(4):
                    q_sb = pq.tile([128, HPC, 512], mdt, tag="q")
                    nc.sync.dma_start(
                        q_sb[:],
                        q_sp[:, qb * 512 : (qb + 1) * 512].rearrange(
                            "(hp p) t -> p hp t", p=128
                        ),
                    )
                    nkb = 4 * qb + 4
                    for h in range(HPC):
                        q_ap = q_sb[:, h, :]
                        # pass 1: all score matmuls + exp (PE runs ahead of ACT)
                        Es = []
                        for kb in range(nkb):
                            s_ps = ps_s.tile([128, 512], f32, tag="s")
                            nc.tensor.matmul(
                                s_ps[:],
                                lhsT=kT_sb[:, h, kb * 128 : (kb + 1) * 128],
                                rhs=q_ap,
                                start=True,
                                stop=True,
                            )
                            E = pE.tile([128, 512], mdt, tag="E")
                            nc.scalar.activation(E[:], s_ps[:], AF.Exp, scale=SCALE)
                            t = kb - 4 * qb
                            if t >= 0:
                                nc.vector.tensor_tensor(
                                    E[:], E[:], masks_sb[:, t, :], OP.mult
                                )
                            Es.append(E)
                        sp_ps = ps_s.tile([PFX, 512], f32, tag="s")
                        nc.tensor.matmul(
                            sp_ps[:], lhsT=pkT_sb[:, h, :], rhs=q_ap, start=True, stop=True
                        )
                        EP = pE.tile([PFX, 512], mdt, tag="E")
                        nc.scalar.activation(EP[:], sp_ps[:], AF.Exp, scale=SCALE)
                        # pass 2: PV + denominator accumulation
                        pv_ps = ps_pv.tile([128, 512], f32, tag="pv")
                        den_ps = ps_den.tile([1, 512], f32, tag="den")
                        for kb in range(nkb):
                            nc.tensor.matmul(
                                pv_ps[:],
                                lhsT=v_sb[:, kb, h * 128 : (h + 1) * 128],
                                rhs=Es[kb][:],
                                start=(kb == 0),
                                stop=(kb == nkb - 1),
                            )
                            nc.tensor.matmul(
                                den_ps[:],
                                lhsT=ones_sb[:],
                                rhs=Es[kb][:],
                                start=(kb == 0),
                                stop=(kb == nkb - 1),
                            )
                        pvP_ps = ps_pv.tile([128, 512], f32, tag="pv")
                        nc.tensor.matmul(
                            pvP_ps[:],
                            lhsT=pv_sb[:, h * 128 : (h + 1) * 128],
                            rhs=EP[:],
                            start=True,
                            stop=True,
                        )
                        denP_ps = ps_den.tile([1, 512], f32, tag="den")
                        nc.tensor.matmul(
                            denP_ps[:],
                            lhsT=ones_sb[0:PFX, :],
                            rhs=EP[:],
                            start=True,
                            stop=True,
                        )
                        # combine: att = pv/den + g * pvP/denP
                        r1 = pc.tile([1, 512], f32, tag="r1")
                        nc.vector.reciprocal(r1[:], den_ps[:])
                        r2 = pc.tile([1, 512], f32, tag="r2")
                        nc.vector.reciprocal(r2[:], denP_ps[:])
                        nc.vector.tensor_scalar_mul(r2[:], r2[:], g_sb[0:1, h : h + 1])
                        rb1 = pc.tile([128, 512], f32, tag="rb1")
                        nc.gpsimd.partition_broadcast(rb1[:], r1[:])
                        rb2 = pc.tile([128, 512], f32, tag="rb2")
                        nc.gpsimd.partition_broadcast(rb2[:], r2[:])
                        t1 = pc.tile([128, 512], f32, tag="t1")
                        nc.vector.tensor_tensor(t1[:], pv_ps[:], rb1[:], OP.mult)
                        t2 = pc.tile([128, 512], f32, tag="t2")
                        nc.vector.tensor_tensor(t2[:], pvP_ps[:], rb2[:], OP.mult)
                        nc.vector.tensor_tensor(
                            attT_sb[:, h, qb * 512 : (qb + 1) * 512],
                            t1[:],
                            t2[:],
                            OP.add,
                        )
                    # fused phase 3 for this q-block
                    for nb in range(D // 512):
                        wo_sb = pwo.tile([128, COLS // 128, 512], mdt, tag="wo")
                        nc.sync.dma_start(
                            wo_sb[:],
                            wo_d[:, nb * 512 : (nb + 1) * 512].rearrange(
                                "(kc p) d -> p kc d", p=128
                            ),
                        )
                        for mt in range(qb * 4, (qb + 1) * 4):
                            ps = ps3.tile([128, 512], f32, tag="mm3")
                            for kc in range(COLS // 128):
                                nc.tensor.matmul(
                                    ps[:],
                                    lhsT=attT_sb[:, kc, mt * 128 : (mt + 1) * 128],
                                    rhs=wo_sb[:, kc, :],
                                    start=(kc == 0),
                                    stop=(kc == COLS // 128 - 1),
                                )
                            o = pout.tile([128, 512], f32, tag="o3")
                            nc.scalar.activation(o[:], ps[:], AF.Copy)
                            nc.sync.dma_start(
                                out_d[
                                    mt * 128 : (mt + 1) * 128,
                                    nb * 512 : (nb + 1) * 512,
                                ],
                                o[:],
                            )

    nc.compile()
    return nc


def _host_inputs(x, freqs_cos, freqs_sin, prefix, prefix_gate, wq, wk, wv, wo):
    import ml_dtypes

    bf16 = ml_dtypes.bfloat16

    x = np.asarray(x, np.float32)
    freqs_cos = np.asarray(freqs_cos, np.float32)
    freqs_sin = np.asarray(freqs_sin, np.float32)
    prefix = np.asarray(prefix, np.float32)
    prefix_gate = np.asarray(prefix_gate, np.float32)
    wq = np.asarray(wq, np.float32)
    wk = np.asarray(wk, np.float32)
    wv = np.asarray(wv, np.float32)
    wo = np.asarray(wo, np.float32)

    cosS = np.ascontiguousarray(np.tile(freqs_cos, (1, 2))).astype(bf16)
    sinS = np.ascontiguousarray(np.tile(freqs_sin, (1, 2))).astype(bf16)
    ii = np.arange(128)[:, None, None]
    tt = np.arange(4)[None, :, None]
    jj = np.arange(512)[None, None, :]
    masks = (jj >= ii + 128 * tt).astype(bf16)
    ones = np.ones((128, 1), bf16)
    eye = np.eye(128, dtype=bf16)
    pfT = np.ascontiguousarray(prefix[0].T).astype(bf16)
    g = np.tanh(prefix_gate)

    xTs = [np.ascontiguousarray(x[b].T).astype(bf16) for b in range(B)]
    in_maps = []
    for c in range(NCORES):
        b, gi = divmod(c, CPB)
        cols = slice(gi * COLS, (gi + 1) * COLS)
        wqkv = np.ascontiguousarray(
            np.concatenate([wq[:, cols], wk[:, cols], wv[:, cols]], axis=1)
        ).astype(bf16)
        in_maps.append(
            dict(
                xT=xTs[b],
                wqkv=wqkv,
                wo=np.ascontiguousarray(wo[cols, :]).astype(bf16),
                pfT=pfT,
                cosS=cosS,
                sinS=sinS,
                masks=masks,
                ones=ones,
                eye=eye,
                g=np.ascontiguousarray(g[None, gi * HPC : (gi + 1) * HPC]),
            )
        )
    return in_maps


def _run(inputs, trace=False, mm_fp32r=True):
    from concourse.bass_utils import run_bass_kernel_spmd

    key = ("nc", mm_fp32r)
    if key not in _CACHE:
        _CACHE[key] = _build(mm_fp32r)
    nc = _CACHE[key]
    in_maps = _host_inputs(
        inputs["x"],
        inputs["freqs_cos"],
        inputs["freqs_sin"],
        inputs["prefix"],
        inputs["prefix_gate"],
        inputs["wq"],
        inputs["wk"],
        inputs["wv"],
        inputs["wo"],
    )
    res = run_bass_kernel_spmd(nc, in_maps, list(range(NCORES)), trace=trace)
    parts = [res.results[c]["out"] for c in range(NCORES)]
    out = np.stack(
        [
            parts[0] + parts[1] + parts[2] + parts[3],
            parts[4] + parts[5] + parts[6] + parts[7],
        ],
        axis=0,
    ).astype(np.float32)
    return out, res


def kernel(**inputs) -> np.ndarray:
    out, _ = _run(inputs, trace=False)
    return out
